# revision 6
# baseline (speedup 1.0000x reference)
"""Trainium2 Bass kernel for CustomGraphSAGEModel (2-chain GraphSAGE + final SAGE).

Strategy (8 NeuronCores, SPMD):
  - Nodes block-sharded: core k owns rows [k*6250, (k+1)*6250).
  - Edges bucketed by (dst 128-row window, src half for int16 dma_gather
    indexing), sorted by src, padded to a chunk structure shared by all cores.
  - Aggregation: dma_gather (4 SWDGE queues) fetches neighbor rows; per
    128-edge chunk a scaled one-hot S[e,r] = (dst_local==r)*inv_deg is built on
    DVE and a PE matmul  aggT += X^T @ S  accumulates mean-aggregates in PSUM.
  - Dense math runs in transposed space: hT_new[o,n] = Wl^T aggT + Wr'^T hT + b
    with Wr' = Wr + resW (exact fold), bias applied via ACT per-partition bias.
  - The two chains share layer-0 aggregation and use joint [h1|h2] gather
    tables so one gather pass serves both chains (5 passes total).
  - Per layer, local blocks are transposed back and AllGather'd into the next
    replicated [50000, 256] gather table.
"""
import numpy as np

N = 50000
E = 640000
NCORES = 8
NPC = N // NCORES            # 6250 nodes per core
W = 128                      # dst window rows
HALF = 32768                 # int16 index split
NWIN = (NPC + W - 1) // W    # 49
NPAD = NWIN * W              # 6272
IN_C = 128
HID = 128
OUT_C = 64
MAX_CHUNKS_PER_CALL = 8      # <=1024 rows per dma_gather call


# ---------------------------------------------------------------- host side

def _preprocess(edge_index: np.ndarray):
    src = np.asarray(edge_index[0], dtype=np.int64)
    dst = np.asarray(edge_index[1], dtype=np.int64)
    deg = np.bincount(dst, minlength=N).astype(np.float64)
    inv_deg = np.where(deg > 0, 1.0 / np.maximum(deg, 1.0), 0.0).astype(np.float32)

    counts = np.zeros((NCORES, NWIN, 2), dtype=np.int64)
    buckets = [[[None, None] for _ in range(NWIN)] for _ in range(NCORES)]
    core = dst // NPC
    dl = dst - core * NPC
    win = dl // W
    half = (src >= HALF).astype(np.int64)
    # order all edges once: (core, win, half, src)
    order = np.lexsort((src, half, win, core))
    so, do, co, wo, ho = src[order], dl[order], core[order], win[order], half[order]
    # segment boundaries
    key = ((co * NWIN) + wo) * 2 + ho
    bounds = np.searchsorted(key, np.arange(NCORES * NWIN * 2 + 1))
    for k in range(NCORES):
        for wi in range(NWIN):
            for hi in range(2):
                kk = (k * NWIN + wi) * 2 + hi
                a, b = bounds[kk], bounds[kk + 1]
                buckets[k][wi][hi] = (so[a:b], do[a:b])
                counts[k, wi, hi] = b - a

    nch_wh = np.ceil(counts.max(axis=0) / 128.0).astype(np.int64)  # [NWIN, 2]
    chunk_meta = []
    for wi in range(NWIN):
        for hi in range(2):
            for _ in range(int(nch_wh[wi, hi])):
                chunk_meta.append((wi, hi))
    NCH = len(chunk_meta)

    idx_i16 = np.zeros((NCORES, NCH * 128), dtype=np.int16)
    dst_local = np.full((NCORES, NCH * 128), -1.0, dtype=np.float32)
    invd = np.zeros((NCORES, NCH * 128), dtype=np.float32)
    pos = 0
    for wi in range(NWIN):
        for hi in range(2):
            seg = int(nch_wh[wi, hi]) * 128
            for k in range(NCORES):
                ss, dd = buckets[k][wi][hi]
                n = len(ss)
                idx_i16[k, pos:pos + n] = (ss - (HALF if hi else 0)).astype(np.int16)
                dst_local[k, pos:pos + n] = (dd - wi * W).astype(np.float32)
                invd[k, pos:pos + n] = inv_deg[dd + k * NPC]
            pos += seg

    # pack indices for dma_gather: j -> [j%16, j//16], replicated to 128 parts
    idxp = np.zeros((NCORES, 128, NCH * 8), dtype=np.int16)
    for k in range(NCORES):
        blk = idx_i16[k].reshape(NCH * 8, 16).T  # [16, NCH*8]
        idxp[k] = np.tile(blk, (8, 1))
    # [lane, chunk] layouts
    dstl = np.ascontiguousarray(
        dst_local.reshape(NCORES, NCH, 128).transpose(0, 2, 1))
    invdl = np.ascontiguousarray(
        invd.reshape(NCORES, NCH, 128).transpose(0, 2, 1))
    return {
        "nch_wh": nch_wh,
        "chunk_meta": chunk_meta,
        "NCH": NCH,
        "idxp": idxp,
        "dstl": dstl,
        "invd": invdl,
    }


def _gather_calls(nch_wh):
    """Program order: per window, lo chunks then hi chunks, split into calls of
    <= MAX_CHUNKS_PER_CALL chunks. Returns list of (chunk_start, n_chunks,
    half) in chunk-index space."""
    calls = []
    c0 = 0
    for wi in range(NWIN):
        for hi in range(2):
            n = int(nch_wh[wi, hi])
            p = 0
            while p < n:
                g = min(MAX_CHUNKS_PER_CALL, n - p)
                calls.append((c0 + p, g, hi))
                p += g
            c0 += n
    return calls


# ---------------------------------------------------------------- bass build

def _build_program(pp):
    import concourse.bacc as bacc
    import concourse.mybir as mybir
    from concourse.tile import TileContext
    from concourse.masks import make_identity

    fp32 = mybir.dt.float32
    i16 = mybir.dt.int16
    i32 = mybir.dt.int32
    AF = mybir.ActivationFunctionType
    OP = mybir.AluOpType

    NCH = pp["NCH"]
    nch_wh = pp["nch_wh"]
    chunk_meta = pp["chunk_meta"]
    calls = _gather_calls(nch_wh)

    nc = bacc.Bacc("TRN2", target_bir_lowering=False, debug=False,
                   num_devices=NCORES, num_swdge_queues=4)

    # ---- I/O
    x_full = nc.dram_tensor("x_full", [N, IN_C], fp32, kind="ExternalInput")
    x_loc = nc.dram_tensor("x_loc", [NPC, IN_C], fp32, kind="ExternalInput")
    idxp = nc.dram_tensor("idxp", [128, NCH * 8], i16, kind="ExternalInput")
    dstl_d = nc.dram_tensor("dstl", [128, NCH], fp32, kind="ExternalInput")
    invd_d = nc.dram_tensor("invd", [128, NCH], fp32, kind="ExternalInput")
    wname = []
    for c in ("c1", "c2"):
        for L in range(4):
            wname += [f"{c}_wl{L}", f"{c}_wr{L}"]
    wts_d = {n: nc.dram_tensor(n, [HID, HID], fp32, kind="ExternalInput")
             for n in wname}
    bias_d = {f"{c}_b{L}": nc.dram_tensor(f"{c}_b{L}", [HID, 1], fp32,
                                          kind="ExternalInput")
              for c in ("c1", "c2") for L in range(4)}
    fwl_d = nc.dram_tensor("f_wl", [2 * HID, OUT_C], fp32, kind="ExternalInput")
    fwr_d = nc.dram_tensor("f_wr", [2 * HID, OUT_C], fp32, kind="ExternalInput")
    fb_d = nc.dram_tensor("f_b", [OUT_C, 1], fp32, kind="ExternalInput")
    y = nc.dram_tensor("y", [NPC, OUT_C], fp32, kind="ExternalOutput")

    with TileContext(nc) as tc:
        with (
            tc.tile_pool(name="const", bufs=1) as cpool,
            tc.tile_pool(name="ht", bufs=1) as hpool,
            tc.tile_pool(name="x0", bufs=3) as x0pool,
            tc.tile_pool(name="xj", bufs=3) as xjpool,
            tc.tile_pool(name="s", bufs=6) as spool,
            tc.tile_pool(name="aggsb", bufs=4) as asbpool,
            tc.tile_pool(name="hnew", bufs=4) as hnpool,
            tc.tile_pool(name="stage", bufs=3) as stpool,
            tc.tile_pool(name="smax", bufs=4) as smpool,
            tc.tile_pool(name="psA", bufs=2, space="PSUM") as psA,
            tc.tile_pool(name="psB", bufs=2, space="PSUM") as psB,
            tc.tile_pool(name="psC", bufs=2, space="PSUM") as psC,
            tc.tile_pool(name="psD", bufs=2, space="PSUM") as psD,
            tc.tile_pool(name="dram", bufs=1, space="DRAM") as dpool,
        ):
            # ---- constants / parameters in SBUF
            ident = cpool.tile([128, 128], fp32)
            make_identity(nc, ident[:])
            iota_i = cpool.tile([128, 128], i32)
            nc.gpsimd.iota(iota_i[:], pattern=[[1, 128]], base=0,
                           channel_multiplier=0)
            iota_f = cpool.tile([128, 128], fp32)
            nc.vector.tensor_copy(out=iota_f[:], in_=iota_i[:])

            idx_sb = cpool.tile([128, NCH * 8], i16)
            nc.sync.dma_start(out=idx_sb[:], in_=idxp[:])
            dstl_sb = cpool.tile([128, NCH], fp32)
            nc.sync.dma_start(out=dstl_sb[:], in_=dstl_d[:])
            invd_sb = cpool.tile([128, NCH], fp32)
            nc.sync.dma_start(out=invd_sb[:], in_=invd_d[:])

            wts = {}
            for n, d in wts_d.items():
                t = cpool.tile([HID, HID], fp32, tag=n, name=n)
                nc.sync.dma_start(out=t[:], in_=d[:])
                wts[n] = t
            bias = {}
            for n, d in bias_d.items():
                t = cpool.tile([HID, 1], fp32, tag=n, name=n)
                nc.sync.dma_start(out=t[:], in_=d[:])
                bias[n] = t
            fwl = [cpool.tile([HID, OUT_C], fp32, tag=f"f_wl{i}",
                              name=f"fwl{i}") for i in range(2)]
            fwr = [cpool.tile([HID, OUT_C], fp32, tag=f"f_wr{i}",
                              name=f"fwr{i}") for i in range(2)]
            for i in range(2):
                nc.sync.dma_start(out=fwl[i][:],
                                  in_=fwl_d[i * HID:(i + 1) * HID, :])
                nc.sync.dma_start(out=fwr[i][:],
                                  in_=fwr_d[i * HID:(i + 1) * HID, :])
            fb = cpool.tile([OUT_C, 1], fp32, tag="f_b")
            nc.sync.dma_start(out=fb[:], in_=fb_d[:])

            # ---- hT double buffers: [128 feat, NPAD nodes] per chain
            # L0 reads xt for both chains, writes ht1[1], ht2[1];
            # L reads [L%2], writes [(L+1)%2]; final reads [0].
            xt = hpool.tile([128, NPAD], fp32, tag="xt")
            ht1 = [hpool.tile([128, NPAD], fp32, tag=f"ht1_{i}",
                              name=f"ht1_{i}") for i in range(2)]
            ht2 = [hpool.tile([128, NPAD], fp32, tag=f"ht2_{i}",
                              name=f"ht2_{i}") for i in range(2)]

            # transpose x_loc into xt
            for w in range(NWIN):
                rows = min(W, NPC - w * W)
                xin = stpool.tile([128, 128], fp32, tag="xin")
                if rows < W:
                    nc.vector.memset(xin[:], 0.0)
                nc.sync.dma_start(out=xin[:rows, :],
                                  in_=x_loc[w * W:w * W + rows, :])
                tp = psD.tile([128, 128], fp32, tag="tp")
                nc.tensor.transpose(out=tp[:], in_=xin[:], identity=ident[:])
                nc.scalar.activation(out=xt[:, w * W:(w + 1) * W], in_=tp[:],
                                     func=AF.Copy)

            # joint gather tables (replicated, rebuilt per layer via AllGather)
            joint_loc = [dpool.tile([NPC, 2 * HID], fp32, tag=f"jl{L}",
                                    name=f"jl{L}") for L in range(4)]
            joint_full = [dpool.tile([N, 2 * HID], fp32, tag=f"jf{L}",
                                     name=f"jf{L}", addr_space="Shared")
                          for L in range(4)]

            qctr = [0]

            def gather_pass(table_lo, table_hi, feat, xpool, xtag):
                """Issue all gather calls for one pass; returns list mapping
                chunk index -> (tile, slot) for the X tiles."""
                chunk_src = [None] * NCH
                for (c0, g, hi) in calls:
                    xtile = xpool.tile([128, MAX_CHUNKS_PER_CALL, feat], fp32,
                                       tag=xtag)
                    tab = table_hi if hi else table_lo
                    nc.gpsimd.dma_gather(
                        xtile[:, :g, :],
                        tab,
                        idx_sb[:, c0 * 8:(c0 + g) * 8],
                        g * 128,
                        g * 128,
                        feat,
                        queue_num=qctr[0] % 4,
                    )
                    qctr[0] += 1
                    for j in range(g):
                        chunk_src[c0 + j] = (xtile, j)
                return chunk_src

            def build_S(c):
                S = spool.tile([128, W], fp32, tag="S")
                nc.vector.tensor_scalar(
                    out=S[:], in0=iota_f[:, :W],
                    scalar1=dstl_sb[:, c:c + 1],
                    scalar2=invd_sb[:, c:c + 1],
                    op0=OP.is_equal, op1=OP.mult)
                return S

            # chunk ranges per window
            win_c0 = []
            c_acc = 0
            for wi in range(NWIN):
                n_w = int(nch_wh[wi, 0] + nch_wh[wi, 1])
                win_c0.append((c_acc, n_w))
                c_acc += n_w

            def scatter_window(wi, chunk_src, feats):
                """Accumulate aggT for window wi. feats: list of (lo, hi) col
                ranges of X to reduce (one per agg output). Returns list of
                PSUM tiles [128, W]."""
                c0, n_w = win_c0[wi]
                aggs = []
                for ai in range(len(feats)):
                    pool = psA if ai == 0 else psB
                    aggs.append(pool.tile([128, W], fp32, tag=f"agg{ai}",
                                          name=f"agg{ai}"))
                for ci in range(n_w):
                    c = c0 + ci
                    S = build_S(c)
                    xtile, j = chunk_src[c]
                    for ai, (f0, f1) in enumerate(feats):
                        nc.tensor.matmul(
                            out=aggs[ai][:],
                            lhsT=xtile[:, j, f0:f1],
                            rhs=S[:],
                            start=(ci == 0), stop=(ci == n_w - 1))
                return aggs

            def drain(ps_tile, tag):
                sb = asbpool.tile([128, W], fp32, tag=tag)
                nc.scalar.activation(out=sb[:], in_=ps_tile[:], func=AF.Copy)
                return sb

            def dense(wi, agg_sb, root_ht, wl, wr, b, relu):
                """hT_new window tile [128 o, W nodes]."""
                ps = psC.tile([128, W], fp32, tag="dense")
                nc.tensor.matmul(out=ps[:], lhsT=wl[:], rhs=agg_sb[:],
                                 start=True, stop=False)
                nc.tensor.matmul(out=ps[:], lhsT=wr[:],
                                 rhs=root_ht[:, wi * W:(wi + 1) * W],
                                 start=False, stop=True)
                hn = hnpool.tile([128, W], fp32, tag="hnew")
                nc.scalar.activation(out=hn[:], in_=ps[:],
                                     func=AF.Relu if relu else AF.Identity,
                                     bias=b[:, :1])
                return hn

            def store_joint(wi, hn1, hn2, jl):
                rows = min(W, NPC - wi * W)
                stage = stpool.tile([128, 2 * HID], fp32, tag="stage")
                for ci, hn in enumerate((hn1, hn2)):
                    tp = psD.tile([128, 128], fp32, tag="tp")
                    nc.tensor.transpose(out=tp[:], in_=hn[:], identity=ident[:])
                    nc.scalar.activation(
                        out=stage[:, ci * HID:(ci + 1) * HID], in_=tp[:],
                        func=AF.Copy)
                nc.sync.dma_start(out=jl[wi * W:wi * W + rows, :],
                                  in_=stage[:rows, :])

            # ================= pass 0 (layer 0, shared aggregation)
            chunk_src = gather_pass(x_full[0:HALF, :], x_full[HALF:N, :],
                                    IN_C, x0pool, "X0")
            for wi in range(NWIN):
                aggs = scatter_window(wi, chunk_src, [(0, 128)])
                a_sb = drain(aggs[0], "a0")
                hn1 = dense(wi, a_sb, xt, wts["c1_wl0"], wts["c1_wr0"],
                            bias["c1_b0"], True)
                hn2 = dense(wi, a_sb, xt, wts["c2_wl0"], wts["c2_wr0"],
                            bias["c2_b0"], True)
                nc.vector.tensor_copy(out=ht1[1][:, wi * W:(wi + 1) * W],
                                      in_=hn1[:])
                nc.vector.tensor_copy(out=ht2[1][:, wi * W:(wi + 1) * W],
                                      in_=hn2[:])
                store_joint(wi, hn1, hn2, joint_loc[0])
            nc.gpsimd.collective_compute(
                "AllGather", mybir.AluOpType.bypass,
                replica_groups=[list(range(NCORES))],
                ins=[joint_loc[0].opt()], outs=[joint_full[0].opt()])

            # ================= passes 1..3
            for L in range(1, 4):
                rd, wr_ = L % 2, (L + 1) % 2
                jf = joint_full[L - 1]
                chunk_src = gather_pass(jf[0:HALF, :], jf[HALF:N, :],
                                        2 * HID, xjpool, "XJ")
                relu = L < 3
                for wi in range(NWIN):
                    aggs = scatter_window(wi, chunk_src, [(0, 128), (128, 256)])
                    a1 = drain(aggs[0], "a1")
                    a2 = drain(aggs[1], "a2")
                    hn1 = dense(wi, a1, ht1[rd], wts[f"c1_wl{L}"],
                                wts[f"c1_wr{L}"], bias[f"c1_b{L}"], relu)
                    hn2 = dense(wi, a2, ht2[rd], wts[f"c2_wl{L}"],
                                wts[f"c2_wr{L}"], bias[f"c2_b{L}"], relu)
                    nc.vector.tensor_copy(out=ht1[wr_][:, wi * W:(wi + 1) * W],
                                          in_=hn1[:])
                    nc.vector.tensor_copy(out=ht2[wr_][:, wi * W:(wi + 1) * W],
                                          in_=hn2[:])
                    store_joint(wi, hn1, hn2, joint_loc[L])
                nc.gpsimd.collective_compute(
                    "AllGather", mybir.AluOpType.bypass,
                    replica_groups=[list(range(NCORES))],
                    ins=[joint_loc[L].opt()], outs=[joint_full[L].opt()])

            # ================= final pass
            jf = joint_full[3]
            chunk_src = gather_pass(jf[0:HALF, :], jf[HALF:N, :],
                                    2 * HID, xjpool, "XJ")
            for wi in range(NWIN):
                rows = min(W, NPC - wi * W)
                aggs = scatter_window(wi, chunk_src, [(0, 128), (128, 256)])
                a1 = drain(aggs[0], "a1")
                a2 = drain(aggs[1], "a2")
                ps = psC.tile([OUT_C, W], fp32, tag="dense")
                nc.tensor.matmul(out=ps[:], lhsT=fwl[0][:], rhs=a1[:],
                                 start=True, stop=False)
                nc.tensor.matmul(out=ps[:], lhsT=fwl[1][:], rhs=a2[:],
                                 start=False, stop=False)
                nc.tensor.matmul(out=ps[:], lhsT=fwr[0][:],
                                 rhs=ht1[0][:, wi * W:(wi + 1) * W],
                                 start=False, stop=False)
                nc.tensor.matmul(out=ps[:], lhsT=fwr[1][:],
                                 rhs=ht2[0][:, wi * W:(wi + 1) * W],
                                 start=False, stop=True)
                oT = hnpool.tile([OUT_C, W], fp32, tag="oT")
                nc.scalar.activation(out=oT[:], in_=ps[:], func=AF.Identity,
                                     bias=fb[:, :1])
                tp = psD.tile([128, OUT_C], fp32, tag="tp")
                nc.tensor.transpose(out=tp[:, :OUT_C], in_=oT[:, :],
                                    identity=ident[:OUT_C, :OUT_C])
                o_sb = smpool.tile([128, OUT_C], fp32, tag="o_sb")
                nc.scalar.activation(out=o_sb[:], in_=tp[:, :OUT_C],
                                     func=AF.Copy)
                # log_softmax along free dim (OUT_C)
                m = smpool.tile([128, 1], fp32, tag="m")
                nc.vector.tensor_reduce(out=m[:], in_=o_sb[:],
                                        axis=mybir.AxisListType.X, op=OP.max)
                mneg = smpool.tile([128, 1], fp32, tag="mneg")
                nc.vector.tensor_scalar_mul(mneg[:], m[:], -1.0)
                ex = smpool.tile([128, OUT_C], fp32, tag="ex")
                nc.scalar.activation(out=ex[:], in_=o_sb[:], func=AF.Exp,
                                     bias=mneg[:, :1])
                s = smpool.tile([128, 1], fp32, tag="s")
                nc.vector.tensor_reduce(out=s[:], in_=ex[:],
                                        axis=mybir.AxisListType.X, op=OP.add)
                ls = smpool.tile([128, 1], fp32, tag="ls")
                nc.scalar.activation(out=ls[:], in_=s[:], func=AF.Ln)
                xm = smpool.tile([128, OUT_C], fp32, tag="xm")
                nc.scalar.activation(out=xm[:], in_=o_sb[:], func=AF.Identity,
                                     bias=mneg[:, :1])
                res = smpool.tile([128, OUT_C], fp32, tag="res")
                nc.vector.tensor_scalar(out=res[:], in0=xm[:],
                                        scalar1=ls[:, :1], scalar2=None,
                                        op0=OP.subtract)
                nc.sync.dma_start(out=y[wi * W:wi * W + rows, :],
                                  in_=res[:rows, :])

    nc.compile()
    return nc


# ---------------------------------------------------------------- entrypoint

_CACHE = {}


def _get_program_and_maps(inputs):
    edge_index = np.asarray(inputs["edge_index"])
    key = hash(edge_index.tobytes())
    if key not in _CACHE:
        pp = _preprocess(edge_index)
        nc = _build_program(pp)
        _CACHE[key] = (pp, nc)
    pp, nc = _CACHE[key]

    x = np.ascontiguousarray(np.asarray(inputs["x"], dtype=np.float32))

    def g(n):
        return np.asarray(inputs[n], dtype=np.float32)

    common = {"x_full": x, "f_wl": np.ascontiguousarray(g("f_Wl")),
              "f_wr": np.ascontiguousarray(g("f_Wr")),
              "f_b": np.ascontiguousarray(g("f_b").reshape(OUT_C, 1))}
    for c in ("c1", "c2"):
        common[f"{c}_wl0"] = np.ascontiguousarray(g(f"{c}_W0l"))
        common[f"{c}_wr0"] = np.ascontiguousarray(g(f"{c}_W0r"))
        common[f"{c}_b0"] = np.ascontiguousarray(
            g(f"{c}_b0").reshape(HID, 1))
        Wl, Wr, b = g(f"{c}_Wl"), g(f"{c}_Wr"), g(f"{c}_b")
        resW, resb = g(f"{c}_resW"), g(f"{c}_resb")
        for i in range(3):
            common[f"{c}_wl{i+1}"] = np.ascontiguousarray(Wl[i])
            common[f"{c}_wr{i+1}"] = np.ascontiguousarray(Wr[i] + resW[i])
            common[f"{c}_b{i+1}"] = np.ascontiguousarray(
                (b[i] + resb[i]).reshape(HID, 1))

    in_maps = []
    for k in range(NCORES):
        m = dict(common)
        m["x_loc"] = np.ascontiguousarray(x[k * NPC:(k + 1) * NPC])
        m["idxp"] = np.ascontiguousarray(pp["idxp"][k])
        m["dstl"] = np.ascontiguousarray(pp["dstl"][k])
        m["invd"] = np.ascontiguousarray(pp["invd"][k])
        in_maps.append(m)
    return nc, in_maps


def run_on_hw(inputs, trace=False):
    from concourse.bass_utils import run_bass_kernel_spmd
    nc, in_maps = _get_program_and_maps(inputs)
    res = run_bass_kernel_spmd(nc, in_maps, core_ids=list(range(NCORES)),
                               trace=trace)
    out = np.concatenate([res.results[k]["y"] for k in range(NCORES)], axis=0)
    return out, res


def kernel(**inputs) -> np.ndarray:
    out, _ = run_on_hw(inputs, trace=False)
    return out


# revision 8
# speedup vs baseline: 2.3873x; 2.3873x over previous
"""Trainium2 Bass kernel for CustomGraphSAGEModel (2-chain GraphSAGE + final SAGE).

Strategy (8 NeuronCores, SPMD):
  - Nodes block-sharded: core k owns rows [k*6250, (k+1)*6250).
  - Gather tables are stored bf16 and SPLIT IN TWO by within-core row
    (r < 3200 vs r >= 3200) so both tables have < 32768 rows (int16
    dma_gather indices) AND the per-layer AllGather splits in two,
    overlapping the first half with the tail of each pass.
  - Edges bucketed by (dst 128-row window, table half), sorted by source
    address, padded to a chunk structure shared by all cores.
  - Aggregation: dma_gather (4 SWDGE queues) fetches neighbor rows; per
    128-edge chunk a host-precomputed scaled one-hot S[e,r] =
    (dst_local==r)*inv_deg (f16, streamed from DRAM) and a PE matmul
    aggT += X^T @ S accumulate mean-aggregates in PSUM (fp32).
  - Dense math runs fp32 in transposed space: hT_new[o,n] = Wl^T aggT +
    Wr'^T hT + b with Wr' = Wr + resW (exact fold), bias via ACT
    per-partition bias, relu fused in the PSUM->SBUF activation.
  - The two chains share layer-0 aggregation and use joint [h1|h2] gather
    tables so one gather pass serves both chains (5 passes total).
"""
import numpy as np

N = 50000
E = 640000
NCORES = 8
NPC = N // NCORES            # 6250 nodes per core
W = 128                      # dst window rows
NWIN = (NPC + W - 1) // W    # 49
NPAD = NWIN * W              # 6272
T1W = (NWIN + 1) // 2        # windows in table/AG half 1 (25)
T1R = T1W * W                # rows per core in table 1 (3200)
T2R = NPC - T1R              # rows per core in table 2 (3050)
NT1 = NCORES * T1R           # 25600
NT2 = NCORES * T2R           # 24400
IN_C = 128
HID = 128
OUT_C = 64
MAX_CHUNKS_PER_CALL = 8      # <=1024 rows per dma_gather call
SGRP = 8                     # S-matrix chunks per DMA group


# ---------------------------------------------------------------- host side

def _preprocess(edge_index: np.ndarray):
    src = np.asarray(edge_index[0], dtype=np.int64)
    dst = np.asarray(edge_index[1], dtype=np.int64)
    deg = np.bincount(dst, minlength=N).astype(np.float64)
    inv_deg = np.where(deg > 0, 1.0 / np.maximum(deg, 1.0), 0.0).astype(np.float32)

    s_core = src // NPC
    s_row = src - s_core * NPC
    hi = (s_row >= T1R).astype(np.int64)
    tab_idx = np.where(hi == 0, s_core * T1R + s_row,
                       s_core * T2R + (s_row - T1R))

    core = dst // NPC
    dl = dst - core * NPC
    win = dl // W
    order = np.lexsort((tab_idx, hi, win, core))
    to, do, co, wo, ho = (tab_idx[order], dl[order], core[order], win[order],
                          hi[order])
    dsto = dst[order]
    key = ((co * NWIN) + wo) * 2 + ho
    bounds = np.searchsorted(key, np.arange(NCORES * NWIN * 2 + 1))

    counts = (bounds[1:] - bounds[:-1]).reshape(NCORES, NWIN, 2)
    nch_wh = np.ceil(counts.max(axis=0) / 128.0).astype(np.int64)  # [NWIN, 2]
    NCH = int(nch_wh.sum())
    NG = (NCH + SGRP - 1) // SGRP

    idx_i16 = np.zeros((NCORES, NCH * 128), dtype=np.int16)
    dst_local = np.full((NCORES, NCH * 128), -1, dtype=np.int64)
    invd = np.zeros((NCORES, NCH * 128), dtype=np.float32)
    pos = 0
    for wi in range(NWIN):
        for hI in range(2):
            seg = int(nch_wh[wi, hI]) * 128
            for k in range(NCORES):
                kk = (k * NWIN + wi) * 2 + hI
                a, b = bounds[kk], bounds[kk + 1]
                n = b - a
                idx_i16[k, pos:pos + n] = to[a:b].astype(np.int16)
                dst_local[k, pos:pos + n] = do[a:b] - wi * W
                invd[k, pos:pos + n] = inv_deg[dsto[a:b]]
            pos += seg
    assert pos == NCH * 128

    # pack indices for dma_gather: j -> [j%16, j//16], replicated to 128 parts
    idxp = np.zeros((NCORES, 128, NCH * 8), dtype=np.int16)
    for k in range(NCORES):
        blk = idx_i16[k].reshape(NCH * 8, 16).T
        idxp[k] = np.tile(blk, (8, 1))

    # host-built scaled one-hot S, f16, grouped [NG, 128, SGRP*W]
    smat = np.zeros((NCORES, NG, 128, SGRP * W), dtype=np.float16)
    lanes = np.arange(128)
    dl2 = dst_local.reshape(NCORES, NCH, 128)
    iv2 = invd.reshape(NCORES, NCH, 128)
    for k in range(NCORES):
        S = np.zeros((NCH, 128, W), dtype=np.float32)
        for c in range(NCH):
            d = dl2[k, c]
            m = d >= 0
            S[c, lanes[m], d[m]] = iv2[k, c, m]
        Sp = np.zeros((NG * SGRP, 128, W), dtype=np.float32)
        Sp[:NCH] = S
        smat[k] = Sp.reshape(NG, SGRP, 128, W).transpose(0, 2, 1, 3).reshape(
            NG, 128, SGRP * W).astype(np.float16)

    return {"nch_wh": nch_wh, "NCH": NCH, "NG": NG, "idxp": idxp,
            "smat": smat}


def _gather_calls(nch_wh):
    calls = []
    c0 = 0
    for wi in range(NWIN):
        for hI in range(2):
            n = int(nch_wh[wi, hI])
            p = 0
            while p < n:
                g = min(MAX_CHUNKS_PER_CALL, n - p)
                calls.append((c0 + p, g, hI))
                p += g
            c0 += n
    return calls


def _x_tables(x):
    xb = x.astype(np.float16)
    x3 = xb.reshape(NCORES, NPC, IN_C)
    xt1 = np.ascontiguousarray(x3[:, :T1R].reshape(NT1, IN_C))
    xt2 = np.ascontiguousarray(x3[:, T1R:].reshape(NT2, IN_C))
    return xt1, xt2


# ---------------------------------------------------------------- bass build

def _build_program(pp):
    import concourse.bacc as bacc
    import concourse.mybir as mybir
    from concourse.tile import TileContext
    from concourse.masks import make_identity

    fp32 = mybir.dt.float32
    f16 = mybir.dt.float16
    i16 = mybir.dt.int16
    AF = mybir.ActivationFunctionType
    OP = mybir.AluOpType

    NCH = pp["NCH"]
    NG = pp["NG"]
    nch_wh = pp["nch_wh"]
    calls = _gather_calls(nch_wh)

    nc = bacc.Bacc("TRN2", target_bir_lowering=False, debug=False,
                   num_devices=NCORES, num_swdge_queues=4)

    # ---- I/O
    xt1_d = nc.dram_tensor("xt1", [NT1, IN_C], f16, kind="ExternalInput")
    xt2_d = nc.dram_tensor("xt2", [NT2, IN_C], f16, kind="ExternalInput")
    x_loc = nc.dram_tensor("x_loc", [NPC, IN_C], fp32, kind="ExternalInput")
    idxp = nc.dram_tensor("idxp", [128, NCH * 8], i16, kind="ExternalInput")
    smat_d = nc.dram_tensor("smat", [NG, 128, SGRP * W], f16,
                            kind="ExternalInput")
    wname = []
    for c in ("c1", "c2"):
        for L in range(4):
            wname += [f"{c}_wl{L}", f"{c}_wr{L}"]
    wts_d = {n: nc.dram_tensor(n, [HID, HID], fp32, kind="ExternalInput")
             for n in wname}
    bias_d = {f"{c}_b{L}": nc.dram_tensor(f"{c}_b{L}", [HID, 1], fp32,
                                          kind="ExternalInput")
              for c in ("c1", "c2") for L in range(4)}
    fwl_d = nc.dram_tensor("f_wl", [2 * HID, OUT_C], fp32, kind="ExternalInput")
    fwr_d = nc.dram_tensor("f_wr", [2 * HID, OUT_C], fp32, kind="ExternalInput")
    fb_d = nc.dram_tensor("f_b", [OUT_C, 1], fp32, kind="ExternalInput")
    y = nc.dram_tensor("y", [NPC, OUT_C], fp32, kind="ExternalOutput")

    with TileContext(nc) as tc:
        with (
            tc.tile_pool(name="const", bufs=1) as cpool,
            tc.tile_pool(name="ht", bufs=1) as hpool,
            tc.tile_pool(name="x0", bufs=6) as x0pool,
            tc.tile_pool(name="xj", bufs=6) as xjpool,
            tc.tile_pool(name="sg", bufs=4) as sgpool,
            tc.tile_pool(name="aggsb", bufs=4) as asbpool,
            tc.tile_pool(name="hnew", bufs=4) as hnpool,
            tc.tile_pool(name="stage", bufs=3) as stpool,
            tc.tile_pool(name="smax", bufs=4) as smpool,
            tc.tile_pool(name="psA", bufs=2, space="PSUM") as psA,
            tc.tile_pool(name="psB", bufs=2, space="PSUM") as psB,
            tc.tile_pool(name="psC", bufs=2, space="PSUM") as psC,
            tc.tile_pool(name="psD", bufs=2, space="PSUM") as psD,
            tc.tile_pool(name="dram", bufs=1, space="DRAM") as dpool,
        ):
            # ---- constants / parameters
            ident = cpool.tile([128, 128], fp32)
            make_identity(nc, ident[:])
            idx_sb = cpool.tile([128, NCH * 8], i16)
            nc.sync.dma_start(out=idx_sb[:], in_=idxp[:])
            wts = {}
            for n, d in wts_d.items():
                t = cpool.tile([HID, HID], fp32, tag=n, name=n)
                nc.sync.dma_start(out=t[:], in_=d[:])
                wts[n] = t
            bias = {}
            for n, d in bias_d.items():
                t = cpool.tile([HID, 1], fp32, tag=n, name=n)
                nc.sync.dma_start(out=t[:], in_=d[:])
                bias[n] = t
            fwl = [cpool.tile([HID, OUT_C], fp32, tag=f"f_wl{i}",
                              name=f"fwl{i}") for i in range(2)]
            fwr = [cpool.tile([HID, OUT_C], fp32, tag=f"f_wr{i}",
                              name=f"fwr{i}") for i in range(2)]
            for i in range(2):
                nc.sync.dma_start(out=fwl[i][:],
                                  in_=fwl_d[i * HID:(i + 1) * HID, :])
                nc.sync.dma_start(out=fwr[i][:],
                                  in_=fwr_d[i * HID:(i + 1) * HID, :])
            fb = cpool.tile([OUT_C, 1], fp32, tag="f_b")
            nc.sync.dma_start(out=fb[:], in_=fb_d[:])

            # hT buffers [128 feat, NPAD nodes], fp32.
            # ht1[0] doubles as xT for layer 0 (both chains' root input).
            ht1 = [hpool.tile([128, NPAD], fp32, tag=f"ht1_{i}",
                              name=f"ht1_{i}") for i in range(2)]
            ht2 = [hpool.tile([128, NPAD], fp32, tag=f"ht2_{i}",
                              name=f"ht2_{i}") for i in range(2)]
            xt = ht1[0]

            for w in range(NWIN):
                rows = min(W, NPC - w * W)
                xin = stpool.tile([128, 128], fp32, tag="xin", name="xin")
                if rows < W:
                    nc.vector.memset(xin[:], 0.0)
                nc.sync.dma_start(out=xin[:rows, :],
                                  in_=x_loc[w * W:w * W + rows, :])
                tp = psD.tile([128, 128], fp32, tag="tp", name="tpx")
                nc.tensor.transpose(out=tp[:], in_=xin[:], identity=ident[:])
                nc.scalar.activation(out=xt[:, w * W:(w + 1) * W], in_=tp[:],
                                     func=AF.Copy)

            # joint gather tables (f16), split in two AllGather halves
            joint_loc = [dpool.tile([NPC, 2 * HID], f16, tag=f"jl{L}",
                                    name=f"jl{L}") for L in range(4)]
            jt1 = [dpool.tile([NT1, 2 * HID], f16, tag=f"jt1_{L}",
                              name=f"jt1_{L}", addr_space="Shared")
                   for L in range(4)]
            jt2 = [dpool.tile([NT2, 2 * HID], f16, tag=f"jt2_{L}",
                              name=f"jt2_{L}", addr_space="Shared")
                   for L in range(4)]

            qctr = [0]

            def gather_pass(table_lo, table_hi, feat, xpool, xtag):
                chunk_src = [None] * NCH
                for (c0, g, hI) in calls:
                    xtile = xpool.tile([128, MAX_CHUNKS_PER_CALL, feat], f16,
                                       tag=xtag, name=xtag)
                    tab = table_hi if hI else table_lo
                    nc.gpsimd.dma_gather(
                        xtile[:, :g, :], tab,
                        idx_sb[:, c0 * 8:(c0 + g) * 8],
                        g * 128, g * 128, feat,
                        queue_num=qctr[0] % 4)
                    qctr[0] += 1
                    for j in range(g):
                        chunk_src[c0 + j] = (xtile, j)
                return chunk_src

            def load_sgroup(g):
                sg = sgpool.tile([128, SGRP * W], f16, tag="sg", name="sg")
                nc.sync.dma_start(out=sg[:], in_=smat_d[g, :, :])
                return sg

            win_c0 = []
            c_acc = 0
            for wi in range(NWIN):
                n_w = int(nch_wh[wi, 0] + nch_wh[wi, 1])
                win_c0.append((c_acc, n_w))
                c_acc += n_w

            def scatter_window(wi, chunk_src, sgs, feats):
                c0, n_w = win_c0[wi]
                aggs = []
                for ai in range(len(feats)):
                    pool = psA if ai == 0 else psB
                    aggs.append(pool.tile([128, W], fp32, tag=f"agg{ai}",
                                          name=f"agg{ai}"))
                for ci in range(n_w):
                    c = c0 + ci
                    g = c // SGRP
                    if sgs[g] is None:
                        sgs[g] = load_sgroup(g)
                    S = sgs[g][:, (c % SGRP) * W:(c % SGRP + 1) * W]
                    xtile, j = chunk_src[c]
                    for ai, (f0, f1) in enumerate(feats):
                        nc.tensor.matmul(
                            out=aggs[ai][:], lhsT=xtile[:, j, f0:f1], rhs=S,
                            start=(ci == 0), stop=(ci == n_w - 1))
                return aggs

            def drain(ps_tile, tag):
                sb = asbpool.tile([128, W], fp32, tag=tag, name=tag)
                nc.scalar.activation(out=sb[:], in_=ps_tile[:], func=AF.Copy)
                return sb

            def dense(wi, agg_sb, root_ht, wl, wr, b, relu):
                ps = psC.tile([128, W], fp32, tag="dense", name="dense")
                nc.tensor.matmul(out=ps[:], lhsT=wl[:], rhs=agg_sb[:],
                                 start=True, stop=False)
                nc.tensor.matmul(out=ps[:], lhsT=wr[:],
                                 rhs=root_ht[:, wi * W:(wi + 1) * W],
                                 start=False, stop=True)
                hn = hnpool.tile([128, W], fp32, tag="hnew", name="hn")
                nc.scalar.activation(out=hn[:], in_=ps[:],
                                     func=AF.Relu if relu else AF.Identity,
                                     bias=b[:, :1])
                return hn

            def store_joint(wi, hn1, hn2, jl):
                rows = min(W, NPC - wi * W)
                stage = stpool.tile([128, 2 * HID], f16, tag="stage",
                                    name="stage")
                for ci, hn in enumerate((hn1, hn2)):
                    tp = psD.tile([128, 128], fp32, tag="tp", name="tpj")
                    nc.tensor.transpose(out=tp[:], in_=hn[:], identity=ident[:])
                    nc.scalar.activation(
                        out=stage[:, ci * HID:(ci + 1) * HID], in_=tp[:],
                        func=AF.Copy)
                nc.sync.dma_start(out=jl[wi * W:wi * W + rows, :],
                                  in_=stage[:rows, :])

            def allgather(jl, tout, part):
                if part == 1:
                    ins_ = jl[0:T1R, :]
                else:
                    ins_ = jl[T1R:NPC, :]
                nc.gpsimd.collective_compute(
                    "AllGather", mybir.AluOpType.bypass,
                    replica_groups=[list(range(NCORES))],
                    ins=[ins_], outs=[tout.opt()])

            # ================= pass 0 (layer 0, shared aggregation)
            chunk_src = gather_pass(xt1_d[:], xt2_d[:], IN_C, x0pool, "X0")
            sgs = [None] * NG
            for wi in range(NWIN):
                aggs = scatter_window(wi, chunk_src, sgs, [(0, 128)])
                a_sb = drain(aggs[0], "a0")
                hn1 = dense(wi, a_sb, xt, wts["c1_wl0"], wts["c1_wr0"],
                            bias["c1_b0"], True)
                hn2 = dense(wi, a_sb, xt, wts["c2_wl0"], wts["c2_wr0"],
                            bias["c2_b0"], True)
                nc.vector.tensor_copy(out=ht1[1][:, wi * W:(wi + 1) * W],
                                      in_=hn1[:])
                nc.vector.tensor_copy(out=ht2[1][:, wi * W:(wi + 1) * W],
                                      in_=hn2[:])
                store_joint(wi, hn1, hn2, joint_loc[0])
                if wi == T1W - 1:
                    allgather(joint_loc[0], jt1[0], 1)
            allgather(joint_loc[0], jt2[0], 2)

            # ================= passes 1..3
            for L in range(1, 4):
                rd, wr_ = L % 2, (L + 1) % 2
                chunk_src = gather_pass(jt1[L - 1][:], jt2[L - 1][:],
                                        2 * HID, xjpool, "XJ")
                sgs = [None] * NG
                relu = L < 3
                for wi in range(NWIN):
                    aggs = scatter_window(wi, chunk_src, sgs,
                                          [(0, 128), (128, 256)])
                    a1 = drain(aggs[0], "a1")
                    a2 = drain(aggs[1], "a2")
                    hn1 = dense(wi, a1, ht1[rd], wts[f"c1_wl{L}"],
                                wts[f"c1_wr{L}"], bias[f"c1_b{L}"], relu)
                    hn2 = dense(wi, a2, ht2[rd], wts[f"c2_wl{L}"],
                                wts[f"c2_wr{L}"], bias[f"c2_b{L}"], relu)
                    nc.vector.tensor_copy(out=ht1[wr_][:, wi * W:(wi + 1) * W],
                                          in_=hn1[:])
                    nc.vector.tensor_copy(out=ht2[wr_][:, wi * W:(wi + 1) * W],
                                          in_=hn2[:])
                    store_joint(wi, hn1, hn2, joint_loc[L])
                    if wi == T1W - 1:
                        allgather(joint_loc[L], jt1[L], 1)
                allgather(joint_loc[L], jt2[L], 2)

            # ================= final pass
            chunk_src = gather_pass(jt1[3][:], jt2[3][:], 2 * HID, xjpool, "XJ")
            sgs = [None] * NG
            for wi in range(NWIN):
                rows = min(W, NPC - wi * W)
                aggs = scatter_window(wi, chunk_src, sgs,
                                      [(0, 128), (128, 256)])
                a1 = drain(aggs[0], "a1")
                a2 = drain(aggs[1], "a2")
                ps = psC.tile([OUT_C, W], fp32, tag="dense", name="densef")
                nc.tensor.matmul(out=ps[:], lhsT=fwl[0][:], rhs=a1[:],
                                 start=True, stop=False)
                nc.tensor.matmul(out=ps[:], lhsT=fwl[1][:], rhs=a2[:],
                                 start=False, stop=False)
                nc.tensor.matmul(out=ps[:], lhsT=fwr[0][:],
                                 rhs=ht1[0][:, wi * W:(wi + 1) * W],
                                 start=False, stop=False)
                nc.tensor.matmul(out=ps[:], lhsT=fwr[1][:],
                                 rhs=ht2[0][:, wi * W:(wi + 1) * W],
                                 start=False, stop=True)
                oT = hnpool.tile([OUT_C, W], fp32, tag="oT", name="oT")
                nc.scalar.activation(out=oT[:], in_=ps[:], func=AF.Identity,
                                     bias=fb[:, :1])
                tp = psD.tile([128, OUT_C], fp32, tag="tp", name="tpf")
                nc.tensor.transpose(out=tp[:, :OUT_C], in_=oT[:, :],
                                    identity=ident[:OUT_C, :OUT_C])
                o_sb = smpool.tile([128, OUT_C], fp32, tag="o_sb", name="osb")
                nc.scalar.activation(out=o_sb[:], in_=tp[:, :OUT_C],
                                     func=AF.Copy)
                m = smpool.tile([128, 1], fp32, tag="m", name="m")
                nc.vector.tensor_reduce(out=m[:], in_=o_sb[:],
                                        axis=mybir.AxisListType.X, op=OP.max)
                mneg = smpool.tile([128, 1], fp32, tag="mneg", name="mneg")
                nc.vector.tensor_scalar_mul(mneg[:], m[:], -1.0)
                ex = smpool.tile([128, OUT_C], fp32, tag="ex", name="ex")
                nc.scalar.activation(out=ex[:], in_=o_sb[:], func=AF.Exp,
                                     bias=mneg[:, :1])
                s = smpool.tile([128, 1], fp32, tag="s", name="s")
                nc.vector.tensor_reduce(out=s[:], in_=ex[:],
                                        axis=mybir.AxisListType.X, op=OP.add)
                ls = smpool.tile([128, 1], fp32, tag="ls", name="ls")
                nc.scalar.activation(out=ls[:], in_=s[:], func=AF.Ln)
                xm = smpool.tile([128, OUT_C], fp32, tag="xm", name="xm")
                nc.scalar.activation(out=xm[:], in_=o_sb[:], func=AF.Identity,
                                     bias=mneg[:, :1])
                res = smpool.tile([128, OUT_C], fp32, tag="res", name="res")
                nc.vector.tensor_scalar(out=res[:], in0=xm[:],
                                        scalar1=ls[:, :1], scalar2=None,
                                        op0=OP.subtract)
                nc.sync.dma_start(out=y[wi * W:wi * W + rows, :],
                                  in_=res[:rows, :])

    nc.compile()
    return nc


# ---------------------------------------------------------------- entrypoint

_CACHE = {}


def _get_program_and_maps(inputs):
    edge_index = np.asarray(inputs["edge_index"])
    key = hash(edge_index.tobytes())
    if key not in _CACHE:
        pp = _preprocess(edge_index)
        nc = _build_program(pp)
        _CACHE[key] = (pp, nc)
    pp, nc = _CACHE[key]

    x = np.ascontiguousarray(np.asarray(inputs["x"], dtype=np.float32))
    xt1, xt2 = _x_tables(x)

    def g(n):
        return np.asarray(inputs[n], dtype=np.float32)

    common = {"xt1": xt1, "xt2": xt2,
              "f_wl": np.ascontiguousarray(g("f_Wl")),
              "f_wr": np.ascontiguousarray(g("f_Wr")),
              "f_b": np.ascontiguousarray(g("f_b").reshape(OUT_C, 1))}
    for c in ("c1", "c2"):
        common[f"{c}_wl0"] = np.ascontiguousarray(g(f"{c}_W0l"))
        common[f"{c}_wr0"] = np.ascontiguousarray(g(f"{c}_W0r"))
        common[f"{c}_b0"] = np.ascontiguousarray(g(f"{c}_b0").reshape(HID, 1))
        Wl, Wr, b = g(f"{c}_Wl"), g(f"{c}_Wr"), g(f"{c}_b")
        resW, resb = g(f"{c}_resW"), g(f"{c}_resb")
        for i in range(3):
            common[f"{c}_wl{i+1}"] = np.ascontiguousarray(Wl[i])
            common[f"{c}_wr{i+1}"] = np.ascontiguousarray(Wr[i] + resW[i])
            common[f"{c}_b{i+1}"] = np.ascontiguousarray(
                (b[i] + resb[i]).reshape(HID, 1))

    in_maps = []
    for k in range(NCORES):
        m = dict(common)
        m["x_loc"] = np.ascontiguousarray(x[k * NPC:(k + 1) * NPC])
        m["idxp"] = np.ascontiguousarray(pp["idxp"][k])
        m["smat"] = np.ascontiguousarray(pp["smat"][k])
        in_maps.append(m)
    return nc, in_maps


def run_on_hw(inputs, trace=False):
    from concourse.bass_utils import run_bass_kernel_spmd
    nc, in_maps = _get_program_and_maps(inputs)
    res = run_bass_kernel_spmd(nc, in_maps, core_ids=list(range(NCORES)),
                               trace=trace)
    out = np.concatenate([res.results[k]["y"] for k in range(NCORES)], axis=0)
    return out, res


def kernel(**inputs) -> np.ndarray:
    out, _ = run_on_hw(inputs, trace=False)
    return out


# revision 9
# speedup vs baseline: 2.4459x; 1.0245x over previous
"""Trainium2 Bass kernel for CustomGraphSAGEModel (2-chain GraphSAGE + final SAGE).

Strategy (8 NeuronCores, SPMD):
  - Nodes block-sharded: core k owns rows [k*6250, (k+1)*6250).
  - Gather tables are stored bf16 and SPLIT IN TWO by within-core row
    (r < 3200 vs r >= 3200) so both tables have < 32768 rows (int16
    dma_gather indices) AND the per-layer AllGather splits in two,
    overlapping the first half with the tail of each pass.
  - Edges bucketed by (dst 128-row window, table half), sorted by source
    address, padded to a chunk structure shared by all cores.
  - Aggregation: dma_gather (4 SWDGE queues) fetches neighbor rows; per
    128-edge chunk a host-precomputed scaled one-hot S[e,r] =
    (dst_local==r)*inv_deg (f16, streamed from DRAM) and a PE matmul
    aggT += X^T @ S accumulate mean-aggregates in PSUM (fp32).
  - Dense math runs fp32 in transposed space: hT_new[o,n] = Wl^T aggT +
    Wr'^T hT + b with Wr' = Wr + resW (exact fold), bias via ACT
    per-partition bias, relu fused in the PSUM->SBUF activation.
  - The two chains share layer-0 aggregation and use joint [h1|h2] gather
    tables so one gather pass serves both chains (5 passes total).
"""
import numpy as np

N = 50000
E = 640000
NCORES = 8
NPC = N // NCORES            # 6250 nodes per core
W = 128                      # dst window rows
NWIN = (NPC + W - 1) // W    # 49
NPAD = NWIN * W              # 6272
T1W = (NWIN + 1) // 2        # windows in table/AG half 1 (25)
T1R = T1W * W                # rows per core in table 1 (3200)
T2R = NPC - T1R              # rows per core in table 2 (3050)
NT1 = NCORES * T1R           # 25600
NT2 = NCORES * T2R           # 24400
IN_C = 128
HID = 128
OUT_C = 64
MAX_CHUNKS_PER_CALL = 8      # <=1024 rows per dma_gather call
SGRP = 8                     # S-matrix chunks per DMA group


# ---------------------------------------------------------------- host side

def _preprocess(edge_index: np.ndarray):
    src = np.asarray(edge_index[0], dtype=np.int64)
    dst = np.asarray(edge_index[1], dtype=np.int64)
    deg = np.bincount(dst, minlength=N).astype(np.float64)
    inv_deg = np.where(deg > 0, 1.0 / np.maximum(deg, 1.0), 0.0).astype(np.float32)

    s_core = src // NPC
    s_row = src - s_core * NPC
    hi = (s_row >= T1R).astype(np.int64)
    tab_idx = np.where(hi == 0, s_core * T1R + s_row,
                       s_core * T2R + (s_row - T1R))

    core = dst // NPC
    dl = dst - core * NPC
    win = dl // W
    order = np.lexsort((tab_idx, hi, win, core))
    to, do, co, wo, ho = (tab_idx[order], dl[order], core[order], win[order],
                          hi[order])
    dsto = dst[order]
    key = ((co * NWIN) + wo) * 2 + ho
    bounds = np.searchsorted(key, np.arange(NCORES * NWIN * 2 + 1))

    counts = (bounds[1:] - bounds[:-1]).reshape(NCORES, NWIN, 2)
    nch_wh = np.ceil(counts.max(axis=0) / 128.0).astype(np.int64)  # [NWIN, 2]
    NCH = int(nch_wh.sum())
    NG = (NCH + SGRP - 1) // SGRP

    idx_i16 = np.zeros((NCORES, NCH * 128), dtype=np.int16)
    dst_local = np.full((NCORES, NCH * 128), -1, dtype=np.int64)
    invd = np.zeros((NCORES, NCH * 128), dtype=np.float32)
    pos = 0
    for wi in range(NWIN):
        for hI in range(2):
            seg = int(nch_wh[wi, hI]) * 128
            for k in range(NCORES):
                kk = (k * NWIN + wi) * 2 + hI
                a, b = bounds[kk], bounds[kk + 1]
                n = b - a
                idx_i16[k, pos:pos + n] = to[a:b].astype(np.int16)
                dst_local[k, pos:pos + n] = do[a:b] - wi * W
                invd[k, pos:pos + n] = inv_deg[dsto[a:b]]
            pos += seg
    assert pos == NCH * 128

    # pack indices for dma_gather: j -> [j%16, j//16], replicated to 128 parts
    idxp = np.zeros((NCORES, 128, NCH * 8), dtype=np.int16)
    for k in range(NCORES):
        blk = idx_i16[k].reshape(NCH * 8, 16).T
        idxp[k] = np.tile(blk, (8, 1))

    # host-built scaled one-hot S, f16, grouped [NG, 128, SGRP*W]
    smat = np.zeros((NCORES, NG, 128, SGRP * W), dtype=np.float16)
    lanes = np.arange(128)
    dl2 = dst_local.reshape(NCORES, NCH, 128)
    iv2 = invd.reshape(NCORES, NCH, 128)
    for k in range(NCORES):
        S = np.zeros((NCH, 128, W), dtype=np.float32)
        for c in range(NCH):
            d = dl2[k, c]
            m = d >= 0
            S[c, lanes[m], d[m]] = iv2[k, c, m]
        Sp = np.zeros((NG * SGRP, 128, W), dtype=np.float32)
        Sp[:NCH] = S
        smat[k] = Sp.reshape(NG, SGRP, 128, W).transpose(0, 2, 1, 3).reshape(
            NG, 128, SGRP * W).astype(np.float16)

    return {"nch_wh": nch_wh, "NCH": NCH, "NG": NG, "idxp": idxp,
            "smat": smat}


def _gather_calls(nch_wh):
    calls = []
    c0 = 0
    for wi in range(NWIN):
        for hI in range(2):
            n = int(nch_wh[wi, hI])
            p = 0
            while p < n:
                g = min(MAX_CHUNKS_PER_CALL, n - p)
                calls.append((c0 + p, g, hI))
                p += g
            c0 += n
    return calls


def _x_tables(x):
    xb = x.astype(np.float16)
    x3 = xb.reshape(NCORES, NPC, IN_C)
    xt1 = np.ascontiguousarray(x3[:, :T1R].reshape(NT1, IN_C))
    xt2 = np.ascontiguousarray(x3[:, T1R:].reshape(NT2, IN_C))
    return xt1, xt2


# ---------------------------------------------------------------- bass build

def _build_program(pp):
    import concourse.bacc as bacc
    import concourse.mybir as mybir
    from concourse.tile import TileContext
    from concourse.masks import make_identity

    fp32 = mybir.dt.float32
    f16 = mybir.dt.float16
    i16 = mybir.dt.int16
    AF = mybir.ActivationFunctionType
    OP = mybir.AluOpType

    NCH = pp["NCH"]
    NG = pp["NG"]
    nch_wh = pp["nch_wh"]
    calls = _gather_calls(nch_wh)

    nc = bacc.Bacc("TRN2", target_bir_lowering=False, debug=False,
                   num_devices=NCORES, num_swdge_queues=4)

    # ---- I/O
    xt1_d = nc.dram_tensor("xt1", [NT1, IN_C], f16, kind="ExternalInput")
    xt2_d = nc.dram_tensor("xt2", [NT2, IN_C], f16, kind="ExternalInput")
    x_loc = nc.dram_tensor("x_loc", [NPC, IN_C], fp32, kind="ExternalInput")
    idxp = nc.dram_tensor("idxp", [128, NCH * 8], i16, kind="ExternalInput")
    smat_d = nc.dram_tensor("smat", [NG, 128, SGRP * W], f16,
                            kind="ExternalInput")
    wname = []
    for c in ("c1", "c2"):
        for L in range(4):
            wname += [f"{c}_wl{L}", f"{c}_wr{L}"]
    wts_d = {n: nc.dram_tensor(n, [HID, HID], fp32, kind="ExternalInput")
             for n in wname}
    bias_d = {f"{c}_b{L}": nc.dram_tensor(f"{c}_b{L}", [HID, 1], fp32,
                                          kind="ExternalInput")
              for c in ("c1", "c2") for L in range(4)}
    fwl_d = nc.dram_tensor("f_wl", [2 * HID, OUT_C], fp32, kind="ExternalInput")
    fwr_d = nc.dram_tensor("f_wr", [2 * HID, OUT_C], fp32, kind="ExternalInput")
    fb_d = nc.dram_tensor("f_b", [OUT_C, 1], fp32, kind="ExternalInput")
    y = nc.dram_tensor("y", [NPC, OUT_C], fp32, kind="ExternalOutput")

    with TileContext(nc) as tc:
        with (
            tc.tile_pool(name="const", bufs=1) as cpool,
            tc.tile_pool(name="ht", bufs=1) as hpool,
            tc.tile_pool(name="x0", bufs=6) as x0pool,
            tc.tile_pool(name="xj", bufs=6) as xjpool,
            tc.tile_pool(name="sg", bufs=4) as sgpool,
            tc.tile_pool(name="aggsb", bufs=4) as asbpool,
            tc.tile_pool(name="hnew", bufs=4) as hnpool,
            tc.tile_pool(name="stage", bufs=3) as stpool,
            tc.tile_pool(name="smax", bufs=4) as smpool,
            tc.tile_pool(name="psA", bufs=2, space="PSUM") as psA,
            tc.tile_pool(name="psB", bufs=2, space="PSUM") as psB,
            tc.tile_pool(name="psC", bufs=2, space="PSUM") as psC,
            tc.tile_pool(name="psD", bufs=2, space="PSUM") as psD,
            tc.tile_pool(name="dram", bufs=1, space="DRAM") as dpool,
        ):
            # ---- constants / parameters
            ident = cpool.tile([128, 128], fp32)
            make_identity(nc, ident[:])
            idx_sb = cpool.tile([128, NCH * 8], i16)
            nc.sync.dma_start(out=idx_sb[:], in_=idxp[:])
            wts = {}
            for n, d in wts_d.items():
                t = cpool.tile([HID, HID], fp32, tag=n, name=n)
                nc.sync.dma_start(out=t[:], in_=d[:])
                wts[n] = t
            bias = {}
            for n, d in bias_d.items():
                t = cpool.tile([HID, 1], fp32, tag=n, name=n)
                nc.sync.dma_start(out=t[:], in_=d[:])
                bias[n] = t
            fwl = [cpool.tile([HID, OUT_C], fp32, tag=f"f_wl{i}",
                              name=f"fwl{i}") for i in range(2)]
            fwr = [cpool.tile([HID, OUT_C], fp32, tag=f"f_wr{i}",
                              name=f"fwr{i}") for i in range(2)]
            for i in range(2):
                nc.sync.dma_start(out=fwl[i][:],
                                  in_=fwl_d[i * HID:(i + 1) * HID, :])
                nc.sync.dma_start(out=fwr[i][:],
                                  in_=fwr_d[i * HID:(i + 1) * HID, :])
            fb = cpool.tile([OUT_C, 1], fp32, tag="f_b")
            nc.sync.dma_start(out=fb[:], in_=fb_d[:])

            # hT buffers [128 feat, NPAD nodes], fp32.
            # ht1[0] doubles as xT for layer 0 (both chains' root input).
            ht1 = [hpool.tile([128, NPAD], fp32, tag=f"ht1_{i}",
                              name=f"ht1_{i}") for i in range(2)]
            ht2 = [hpool.tile([128, NPAD], fp32, tag=f"ht2_{i}",
                              name=f"ht2_{i}") for i in range(2)]
            xt = ht1[0]

            for w in range(NWIN):
                rows = min(W, NPC - w * W)
                xin = stpool.tile([128, 128], fp32, tag="xin", name="xin")
                if rows < W:
                    nc.vector.memset(xin[:], 0.0)
                nc.sync.dma_start(out=xin[:rows, :],
                                  in_=x_loc[w * W:w * W + rows, :])
                tp = psD.tile([128, 128], fp32, tag="tp", name="tpx")
                nc.tensor.transpose(out=tp[:], in_=xin[:], identity=ident[:])
                nc.scalar.activation(out=xt[:, w * W:(w + 1) * W], in_=tp[:],
                                     func=AF.Copy)

            # joint gather tables (f16), split in two AllGather halves
            joint_loc = [dpool.tile([NPC, 2 * HID], f16, tag=f"jl{L}",
                                    name=f"jl{L}") for L in range(4)]
            jt1 = [dpool.tile([NT1, 2 * HID], f16, tag=f"jt1_{L}",
                              name=f"jt1_{L}", addr_space="Shared")
                   for L in range(4)]
            jt2 = [dpool.tile([NT2, 2 * HID], f16, tag=f"jt2_{L}",
                              name=f"jt2_{L}", addr_space="Shared")
                   for L in range(4)]

            qctr = [0]

            def gather_pass(table_lo, table_hi, feat, xpool, xtag):
                chunk_src = [None] * NCH
                for (c0, g, hI) in calls:
                    xtile = xpool.tile([128, MAX_CHUNKS_PER_CALL, feat], f16,
                                       tag=xtag, name=xtag)
                    tab = table_hi if hI else table_lo
                    nc.gpsimd.dma_gather(
                        xtile[:, :g, :], tab,
                        idx_sb[:, c0 * 8:(c0 + g) * 8],
                        g * 128, g * 128, feat,
                        queue_num=qctr[0] % 4)
                    qctr[0] += 1
                    for j in range(g):
                        chunk_src[c0 + j] = (xtile, j)
                return chunk_src

            def load_sgroup(g):
                sg = sgpool.tile([128, SGRP * W], f16, tag="sg", name="sg")
                nc.sync.dma_start(out=sg[:], in_=smat_d[g, :, :])
                return sg

            win_c0 = []
            c_acc = 0
            for wi in range(NWIN):
                n_w = int(nch_wh[wi, 0] + nch_wh[wi, 1])
                win_c0.append((c_acc, n_w))
                c_acc += n_w

            def scatter_window(wi, chunk_src, sgs, feats):
                c0, n_w = win_c0[wi]
                aggs = []
                for ai in range(len(feats)):
                    pool = psA if ai == 0 else psB
                    aggs.append(pool.tile([128, W], fp32, tag=f"agg{ai}",
                                          name=f"agg{ai}"))
                for ci in range(n_w):
                    c = c0 + ci
                    g = c // SGRP
                    if sgs[g] is None:
                        sgs[g] = load_sgroup(g)
                    S = sgs[g][:, (c % SGRP) * W:(c % SGRP + 1) * W]
                    xtile, j = chunk_src[c]
                    for ai, (f0, f1) in enumerate(feats):
                        nc.tensor.matmul(
                            out=aggs[ai][:], lhsT=xtile[:, j, f0:f1], rhs=S,
                            start=(ci == 0), stop=(ci == n_w - 1))
                return aggs

            def drain(ps_tile, tag):
                sb = asbpool.tile([128, W], fp32, tag=tag, name=tag)
                nc.scalar.activation(out=sb[:], in_=ps_tile[:], func=AF.Copy)
                return sb

            def dense(wi, agg_sb, root_ht, wl, wr, b, relu, out_ht):
                ps = psC.tile([128, W], fp32, tag="dense", name="dense")
                nc.tensor.matmul(out=ps[:], lhsT=wl[:], rhs=agg_sb[:],
                                 start=True, stop=False)
                nc.tensor.matmul(out=ps[:], lhsT=wr[:],
                                 rhs=root_ht[:, wi * W:(wi + 1) * W],
                                 start=False, stop=True)
                out_sl = out_ht[:, wi * W:(wi + 1) * W]
                nc.scalar.activation(out=out_sl, in_=ps[:],
                                     func=AF.Relu if relu else AF.Identity,
                                     bias=b[:, :1])
                return out_sl

            def store_joint(wi, hn1, hn2, jl):
                rows = min(W, NPC - wi * W)
                stage = stpool.tile([128, 2 * HID], f16, tag="stage",
                                    name="stage")
                for ci, hn in enumerate((hn1, hn2)):
                    tp = psD.tile([128, 128], fp32, tag="tp", name="tpj")
                    nc.tensor.transpose(out=tp[:], in_=hn, identity=ident[:])
                    nc.scalar.activation(
                        out=stage[:, ci * HID:(ci + 1) * HID], in_=tp[:],
                        func=AF.Copy)
                nc.sync.dma_start(out=jl[wi * W:wi * W + rows, :],
                                  in_=stage[:rows, :])

            def allgather(jl, tout, part):
                if part == 1:
                    ins_ = jl[0:T1R, :]
                else:
                    ins_ = jl[T1R:NPC, :]
                nc.gpsimd.collective_compute(
                    "AllGather", mybir.AluOpType.bypass,
                    replica_groups=[list(range(NCORES))],
                    ins=[ins_], outs=[tout.opt()])

            # ================= pass 0 (layer 0, shared aggregation)
            chunk_src = gather_pass(xt1_d[:], xt2_d[:], IN_C, x0pool, "X0")
            sgs = [None] * NG
            for wi in range(NWIN):
                aggs = scatter_window(wi, chunk_src, sgs, [(0, 128)])
                a_sb = drain(aggs[0], "a0")
                hn1 = dense(wi, a_sb, xt, wts["c1_wl0"], wts["c1_wr0"],
                            bias["c1_b0"], True, ht1[1])
                hn2 = dense(wi, a_sb, xt, wts["c2_wl0"], wts["c2_wr0"],
                            bias["c2_b0"], True, ht2[1])
                store_joint(wi, hn1, hn2, joint_loc[0])
                if wi == T1W - 1:
                    allgather(joint_loc[0], jt1[0], 1)
            allgather(joint_loc[0], jt2[0], 2)

            # ================= passes 1..3
            for L in range(1, 4):
                rd, wr_ = L % 2, (L + 1) % 2
                chunk_src = gather_pass(jt1[L - 1][:], jt2[L - 1][:],
                                        2 * HID, xjpool, "XJ")
                sgs = [None] * NG
                relu = L < 3
                for wi in range(NWIN):
                    aggs = scatter_window(wi, chunk_src, sgs,
                                          [(0, 128), (128, 256)])
                    a1 = drain(aggs[0], "a1")
                    a2 = drain(aggs[1], "a2")
                    hn1 = dense(wi, a1, ht1[rd], wts[f"c1_wl{L}"],
                                wts[f"c1_wr{L}"], bias[f"c1_b{L}"], relu,
                                ht1[wr_])
                    hn2 = dense(wi, a2, ht2[rd], wts[f"c2_wl{L}"],
                                wts[f"c2_wr{L}"], bias[f"c2_b{L}"], relu,
                                ht2[wr_])
                    store_joint(wi, hn1, hn2, joint_loc[L])
                    if wi == T1W - 1:
                        allgather(joint_loc[L], jt1[L], 1)
                allgather(joint_loc[L], jt2[L], 2)

            # ================= final pass
            chunk_src = gather_pass(jt1[3][:], jt2[3][:], 2 * HID, xjpool, "XJ")
            sgs = [None] * NG
            for wi in range(NWIN):
                rows = min(W, NPC - wi * W)
                aggs = scatter_window(wi, chunk_src, sgs,
                                      [(0, 128), (128, 256)])
                a1 = drain(aggs[0], "a1")
                a2 = drain(aggs[1], "a2")
                ps = psC.tile([OUT_C, W], fp32, tag="dense", name="densef")
                nc.tensor.matmul(out=ps[:], lhsT=fwl[0][:], rhs=a1[:],
                                 start=True, stop=False)
                nc.tensor.matmul(out=ps[:], lhsT=fwl[1][:], rhs=a2[:],
                                 start=False, stop=False)
                nc.tensor.matmul(out=ps[:], lhsT=fwr[0][:],
                                 rhs=ht1[0][:, wi * W:(wi + 1) * W],
                                 start=False, stop=False)
                nc.tensor.matmul(out=ps[:], lhsT=fwr[1][:],
                                 rhs=ht2[0][:, wi * W:(wi + 1) * W],
                                 start=False, stop=True)
                oT = hnpool.tile([OUT_C, W], fp32, tag="oT", name="oT")
                nc.scalar.activation(out=oT[:], in_=ps[:], func=AF.Identity,
                                     bias=fb[:, :1])
                tp = psD.tile([128, OUT_C], fp32, tag="tp", name="tpf")
                nc.tensor.transpose(out=tp[:, :OUT_C], in_=oT[:, :],
                                    identity=ident[:OUT_C, :OUT_C])
                o_sb = smpool.tile([128, OUT_C], fp32, tag="o_sb", name="osb")
                nc.scalar.activation(out=o_sb[:], in_=tp[:, :OUT_C],
                                     func=AF.Copy)
                m = smpool.tile([128, 1], fp32, tag="m", name="m")
                nc.vector.tensor_reduce(out=m[:], in_=o_sb[:],
                                        axis=mybir.AxisListType.X, op=OP.max)
                mneg = smpool.tile([128, 1], fp32, tag="mneg", name="mneg")
                nc.vector.tensor_scalar_mul(mneg[:], m[:], -1.0)
                ex = smpool.tile([128, OUT_C], fp32, tag="ex", name="ex")
                nc.scalar.activation(out=ex[:], in_=o_sb[:], func=AF.Exp,
                                     bias=mneg[:, :1])
                s = smpool.tile([128, 1], fp32, tag="s", name="s")
                nc.vector.tensor_reduce(out=s[:], in_=ex[:],
                                        axis=mybir.AxisListType.X, op=OP.add)
                ls = smpool.tile([128, 1], fp32, tag="ls", name="ls")
                nc.scalar.activation(out=ls[:], in_=s[:], func=AF.Ln)
                xm = smpool.tile([128, OUT_C], fp32, tag="xm", name="xm")
                nc.scalar.activation(out=xm[:], in_=o_sb[:], func=AF.Identity,
                                     bias=mneg[:, :1])
                res = smpool.tile([128, OUT_C], fp32, tag="res", name="res")
                nc.vector.tensor_scalar(out=res[:], in0=xm[:],
                                        scalar1=ls[:, :1], scalar2=None,
                                        op0=OP.subtract)
                nc.sync.dma_start(out=y[wi * W:wi * W + rows, :],
                                  in_=res[:rows, :])

    nc.compile()
    return nc


# ---------------------------------------------------------------- entrypoint

_CACHE = {}


def _get_program_and_maps(inputs):
    edge_index = np.asarray(inputs["edge_index"])
    key = hash(edge_index.tobytes())
    if key not in _CACHE:
        pp = _preprocess(edge_index)
        nc = _build_program(pp)
        _CACHE[key] = (pp, nc)
    pp, nc = _CACHE[key]

    x = np.ascontiguousarray(np.asarray(inputs["x"], dtype=np.float32))
    xt1, xt2 = _x_tables(x)

    def g(n):
        return np.asarray(inputs[n], dtype=np.float32)

    common = {"xt1": xt1, "xt2": xt2,
              "f_wl": np.ascontiguousarray(g("f_Wl")),
              "f_wr": np.ascontiguousarray(g("f_Wr")),
              "f_b": np.ascontiguousarray(g("f_b").reshape(OUT_C, 1))}
    for c in ("c1", "c2"):
        common[f"{c}_wl0"] = np.ascontiguousarray(g(f"{c}_W0l"))
        common[f"{c}_wr0"] = np.ascontiguousarray(g(f"{c}_W0r"))
        common[f"{c}_b0"] = np.ascontiguousarray(g(f"{c}_b0").reshape(HID, 1))
        Wl, Wr, b = g(f"{c}_Wl"), g(f"{c}_Wr"), g(f"{c}_b")
        resW, resb = g(f"{c}_resW"), g(f"{c}_resb")
        for i in range(3):
            common[f"{c}_wl{i+1}"] = np.ascontiguousarray(Wl[i])
            common[f"{c}_wr{i+1}"] = np.ascontiguousarray(Wr[i] + resW[i])
            common[f"{c}_b{i+1}"] = np.ascontiguousarray(
                (b[i] + resb[i]).reshape(HID, 1))

    in_maps = []
    for k in range(NCORES):
        m = dict(common)
        m["x_loc"] = np.ascontiguousarray(x[k * NPC:(k + 1) * NPC])
        m["idxp"] = np.ascontiguousarray(pp["idxp"][k])
        m["smat"] = np.ascontiguousarray(pp["smat"][k])
        in_maps.append(m)
    return nc, in_maps


def run_on_hw(inputs, trace=False):
    from concourse.bass_utils import run_bass_kernel_spmd
    nc, in_maps = _get_program_and_maps(inputs)
    res = run_bass_kernel_spmd(nc, in_maps, core_ids=list(range(NCORES)),
                               trace=trace)
    out = np.concatenate([res.results[k]["y"] for k in range(NCORES)], axis=0)
    return out, res


def kernel(**inputs) -> np.ndarray:
    out, _ = run_on_hw(inputs, trace=False)
    return out
